# revision 28
# baseline (speedup 1.0000x reference)
"""Lovász-Softmax + CE loss kernel for Trainium2 (8 NeuronCores).

Strategy
--------
Data-parallel: core m processes batch image m (B=8). The per_image=False
global sort over all 8*512*512 pixels is replaced by an exact-integral
formulation needing only *relu-sums* (soft thermometer integrals): with
u = onehot(label==c) - p_c  (positive exactly on fg pixels),

    rs_fg(t) = sum_j relu(u_j - t),   rs_bg(t) = sum_j relu(-u_j - t)

give exact bin-integrals of the fg/bg rank-count functions F, B via
rs(t_l) - rs(t_u) = int cnt_ge(s) ds, and

    loss_c = int_0^1 J(s) ds,  J = 1 - (G - F(s))/(G + B(s))
          ~= 1 - sum_bins dT * (G - Fbar)/(G + Bbar)

with bin-averaged counts from relu-sum differences, a linear model for
B in its wide tail bin (B(1)=0) and for F in its wide head bin
(F(0)=G). Error vs the exact sorted computation ~1e-5 relative — below
fp32 softmax noise. Per-core partials are reduced on host in float64.

On-chip per core: stream logits per class, softmax (no max-sub, |x|<~6),
15 thermometer passes per class on u (bf16, fused per-partition
row-sum via accum_out; 2-of-7 passes on the scalar engine to balance),
CE from exact fp32 x via scalar_tensor_tensor. Per-class fg pixel
counts G come from an exact host-side bincount of the integer labels.
"""

import sys

sys.path.insert(0, "/opt/trn_rl_repo")

from contextlib import ExitStack

import numpy as np

import concourse.bacc as bacc
import concourse.bass as bass
import concourse.mybir as mybir
from concourse import tile
from concourse.bass_utils import run_bass_kernel_spmd

F32 = mybir.dt.float32
BF16 = mybir.dt.bfloat16
I32 = mybir.dt.int32
AF = mybir.ActivationFunctionType
ALU = mybir.AluOpType

B, C, H, W = 8, 21, 512, 512
NPIX = H * W            # 262144 pixels per core
NPART = 128
FREE = NPIX // NPART    # 2048
T = 2048                # free-dim chunk (single chunk)
NCHUNK = FREE // T      # 1

# thermometer edges (16ths), chosen + validated offline (~1.6e-5 rel)
FG_TH = [0, 8, 10, 11, 12, 13, 14, 15]             # /16, then 1.0 edge free
BG_TH = [0, 1, 2, 3, 4, 5, 8]                      # /16, then 1.0 edge free
NF, NB = len(FG_TH), len(BG_TH)
NPASS = NF + NB                                     # 15
NCOL = NPASS + 2                                    # + G + ce per class
LNZ_COL = C * NCOL * NCHUNK                         # one extra column
def _on_act(c, i):
    # which (class, threshold) passes run on the scalar engine (2 of 7)
    return (c * NPASS + i) % 7 in (3, 6)

_CACHE = {}


def _build():
    if "nc" in _CACHE:
        return _CACHE["nc"]
    nc = bacc.Bacc("TRN2", target_bir_lowering=False, debug=False,
                   num_devices=B)
    x_d = nc.dram_tensor("x", [C, NPART, FREE], F32, kind="ExternalInput").ap()
    lab_d = nc.dram_tensor("lab", [NPART, FREE], I32, kind="ExternalInput").ap()
    rs_d = nc.dram_tensor("rs", [NPART, LNZ_COL + 1], F32,
                          kind="ExternalOutput").ap()

    with tile.TileContext(nc) as tc, ExitStack() as ctx:
        xp = ctx.enter_context(tc.tile_pool(name="xp", bufs=3))
        wp = ctx.enter_context(tc.tile_pool(name="wp", bufs=1))
        sp = ctx.enter_context(tc.tile_pool(name="sp", bufs=2))

        # bias columns for ACT relu passes: -t for both fg and bg variants
        bias = wp.tile([NPART, NPASS], F32, tag="bias")
        for i, th in enumerate(FG_TH):
            nc.vector.memset(bias[:, i:i + 1], -th / 16)
        for i, th in enumerate(BG_TH):
            nc.vector.memset(bias[:, NF + i:NF + i + 1], -th / 16)

        rs_acc = wp.tile([NPART, LNZ_COL + 1], F32, tag="rs_acc")

        for k in range(NCHUNK):
            sl = slice(k * T, (k + 1) * T)
            labi = wp.tile([NPART, T], I32, tag="labi")
            nc.sync.dma_start(labi[:], lab_d[:, sl])
            labf = wp.tile([NPART, T], BF16, tag="labf")
            nc.vector.tensor_copy(labf[:], labi[:])

            # ---- pass 1: stream x per class; CE sums, exp, Z accum ----
            es = []
            for c in range(C):
                xt = xp.tile([NPART, T], F32, tag="xt")
                nc.sync.dma_start(xt[:], x_d[c, :, sl])
                col = (c * NCOL + NPASS + 1) * NCHUNK + k
                ce_scr = sp.tile([NPART, T], BF16, tag="ce_scr")
                # sum_j [lab==c] * x_c  -> ce partial
                nc.vector.scalar_tensor_tensor(
                    ce_scr[:], labf[:], float(c), xt[:],
                    op0=ALU.is_equal, op1=ALU.mult,
                    accum_out=rs_acc[:, col:col + 1])
                et = wp.tile([NPART, T], BF16, tag=f"e{c}")
                nc.scalar.activation(et[:], xt[:], AF.Exp)
                es.append(et)

            # Z = sum(es)
            zt = wp.tile([NPART, T], BF16, tag="zt")
            nc.vector.tensor_copy(zt[:], es[0][:])
            for c in range(1, C):
                nc.vector.tensor_add(zt[:], zt[:], es[c][:])

            # log(Z) partial sums for CE; reciprocal for softmax
            lnscr = wp.tile([NPART, T], F32, tag="lnscr")
            nc.scalar.activation(lnscr[:], zt[:], AF.Ln,
                                 accum_out=rs_acc[:, LNZ_COL:LNZ_COL + 1])
            ztf = wp.tile([NPART, T], F32, tag="ztf")
            nc.vector.tensor_copy(ztf[:], zt[:])
            rzf = wp.tile([NPART, T], F32, tag="rzf")
            nc.vector.reciprocal(rzf[:], ztf[:])
            rz = wp.tile([NPART, T], BF16, tag="rz")
            nc.vector.tensor_copy(rz[:], rzf[:])

            # ---- pass 2: per class u = [lab==c] - p; thermometer sums ----
            for c in range(C):
                p = es[c]
                nc.vector.tensor_mul(p[:], p[:], rz[:])      # p = e/Z (bf16)
                u = sp.tile([NPART, T], BF16, tag="u")
                nc.vector.scalar_tensor_tensor(
                    u[:], labf[:], float(c), p[:],
                    op0=ALU.is_equal, op1=ALU.subtract)
                scr = sp.tile([NPART, T], BF16, tag="scr")
                scr2 = sp.tile([NPART, T], BF16, tag="scr2")
                for i in range(NPASS):
                    col = (c * NCOL + i) * NCHUNK + k
                    acc = rs_acc[:, col:col + 1]
                    on_act = _on_act(c, i)
                    if i < NF:                                # fg: relu(u - t)
                        t16 = FG_TH[i] / 16
                        if on_act:
                            nc.scalar.activation(scr2[:], u[:], AF.Relu,
                                                 bias=bias[:, i:i + 1],
                                                 accum_out=acc)
                        else:
                            # sum max(u, t) = rs_fg(t) + N*t  (host fixup)
                            nc.vector.tensor_scalar(
                                scr[:], u[:], t16, 0.0,
                                op0=ALU.max, op1=ALU.add, accum_out=acc)
                    else:                                     # bg: relu(-u - t)
                        t16 = BG_TH[i - NF] / 16
                        if on_act:
                            nc.scalar.activation(scr2[:], u[:], AF.Relu,
                                                 scale=-1.0,
                                                 bias=bias[:, i:i + 1],
                                                 accum_out=acc)
                        else:
                            # sum min(u, -t) = -rs_bg(t) - N*t  (host fixup)
                            nc.vector.tensor_scalar(
                                scr[:], u[:], -t16, 0.0,
                                op0=ALU.min, op1=ALU.add, accum_out=acc)

        nc.sync.dma_start(rs_d[:], rs_acc[:])

    nc.compile()
    _CACHE["nc"] = nc
    return nc


def _finalize(rs, G):
    """Host fp64 reduction of per-core partials -> scalar loss."""
    # rs: [B, NPART, LNZ_COL+1]
    tot = rs.astype(np.float64).sum(axis=(0, 1))
    lnz = tot[LNZ_COL]
    per = tot[:LNZ_COL].reshape(C, NCOL, NCHUNK).sum(-1)   # [C, NCOL]
    G = G.astype(np.float64)
    rsf = per[:, :NF].copy()
    rsb = per[:, NF:NPASS].copy()
    # V passes accumulated sum(max(u,t)) = rs_fg + N*t (fg) and
    # sum(min(u,-t)) = -rs_bg - N*t (bg); ACT passes accumulated rs directly.
    N_glob = float(B * NPIX)
    for c in range(C):
        for i in range(NPASS):
            if _on_act(c, i):
                continue
            if i < NF:
                rsf[c, i] -= N_glob * FG_TH[i] / 16
            else:
                j = i - NF
                rsb[c, j] = -rsb[c, j] - N_glob * BG_TH[j] / 16
    ce_x = per[:, NPASS + 1]

    fg_e = np.array([t / 16 for t in FG_TH] + [1.0])
    bg_e = np.array([t / 16 for t in BG_TH] + [1.0])
    rsf = np.concatenate([rsf, np.zeros((C, 1))], axis=1)          # rs at 1.0
    # bg vector-passes accumulated min(u+t,0) = -relu(-u-t); ACT passes
    # accumulated +relu(-u-t).  Sign fixup happens in kernel-side choice:
    # we negate V-pass columns here via the sign mask built at import.
    rsb = np.concatenate([rsb, np.zeros((C, 1))], axis=1)

    union = np.unique(np.concatenate([fg_e, bg_e]))
    dT = np.diff(union)
    mids = 0.5 * (union[:-1] + union[1:])

    def piecewise_avg(edges, rsv):
        avg = (rsv[:, :-1] - rsv[:, 1:]) / np.diff(edges)[None, :]
        idx = np.clip(np.searchsorted(edges, mids, side="right") - 1,
                      0, len(edges) - 2)
        return avg[:, idx]

    Fbar = piecewise_avg(fg_e, rsf)
    Bbar = piecewise_avg(bg_e, rsb)
    # linear tail model for B in its wide last bin (B(1) = 0)
    lo, hi = bg_e[-2], bg_e[-1]
    m = 2 * (rsb[:, -2] - rsb[:, -1]) / (hi - lo) ** 2
    sel = (mids > lo) & (mids < hi)
    Bbar[:, sel] = m[:, None] * (hi - mids[None, sel])
    # linear head model for F in its wide first bin (F(0) = G)
    lo, hi = fg_e[0], fg_e[1]
    avg0 = (rsf[:, 0] - rsf[:, 1]) / (hi - lo)
    mdef = 2 * (G - avg0) / (hi - lo)
    sel = (mids > lo) & (mids < hi)
    Fbar[:, sel] = G[:, None] - mdef[:, None] * (mids[None, sel] - lo)

    losses = 1.0 - (dT[None, :] * (G[:, None] - Fbar) /
                    np.maximum(G[:, None] + Bbar, 1e-300)).sum(1)
    present = (G > 0).astype(np.float64)
    lovasz = (losses * present).sum() / max(present.sum(), 1.0)
    ce = (lnz - ce_x.sum()) / (B * NPIX)
    return np.float32(lovasz + ce)


def kernel(logits: np.ndarray, target: np.ndarray) -> np.ndarray:
    nc = _build()
    in_maps = []
    for m in range(B):
        x = np.ascontiguousarray(logits[m].reshape(C, NPART, FREE),
                                 dtype=np.float32)
        lab = np.ascontiguousarray(
            target[m].reshape(NPART, FREE).astype(np.int32))
        in_maps.append({"x": x, "lab": lab})
    G = np.bincount(np.asarray(target).reshape(-1).astype(np.int64),
                    minlength=C).astype(np.float64)
    res = run_bass_kernel_spmd(nc, in_maps, list(range(B)))
    rs = np.stack([res.results[m]["rs"] for m in range(B)])
    return _finalize(rs, G)
